# revision 41
# baseline (speedup 1.0000x reference)
"""AttnBlock (GroupNorm -> qkv 1x1 conv -> 8-head attention over 32x32
spatial -> proj 1x1 conv -> residual) on 8 Trainium2 NeuronCores.

Sharding: fully data-parallel, no collectives. Core i handles batch
b = i//2 and query-half s = i%2 (512 of the 1024 spatial positions).
Each core redundantly computes GroupNorm stats plus the full k/v
projections for its batch, then scores/softmax/AV/proj for its query
half. Host concatenates the per-core [512, 512] outputs.

Per-core device program (compute dtype bf16, f32 accumulation):
  - GroupNorm stats via bn_stats/bn_aggr per channel, group-reduced
    across partitions with a tiny selector matmul, expanded back with a
    second matmul; affine folded into one tensor_scalar per tile.
  - Scores are computed transposed, S^T[j,i] = sum_c k[c,j] q[c,i]
    (keys on partitions, K=64 per head), with the two heads of a
    partition-tile row-packed into PE row groups (0,0)/(64,0) so a pair
    costs one N=512 stream.
  - softmax skips the max-subtraction (|scores| <= ~6 for this
    problem's GN'd inputs): one Exp activation per [128,1024] psum with
    the 1/sqrt(64) scale folded in.  The key-dim sum comes free from a
    ones-column appended to v^T in the AV matmul (psum row 64 = Z);
    1/Z = exp(-ln Z) on the scalar engine, broadcast across partitions
    with a tiny expander matmul.
  - proj accumulates k-major so chains start as head-pair outputs
    arrive; residual added from the f32 x half kept on chip.

Toolchain workarounds: the Tile-tail Drain and any instruction carrying
more than one semaphore wait are rejected by this walrus build, so
excess waits are spread onto same-engine NoOps post-schedule.
"""

import os

import numpy as np

import concourse.bass as bass
import concourse.tile as tile
from concourse import mybir
from concourse.bass_utils import run_bass_kernel_spmd
from concourse.vector_clock import ScopedClock

# ---------------------------------------------------------------------------
# walrus workaround: the Tile kernel-tail Drain may carry more sem waits than
# the CTRL instruction encoding allows; spread them over sync-engine NOPs.
_MAX_WAITS_PER_INST = 1


def _patched_drain_and_barrier(self, tick_clock, wait_clock):
    nc = self.nc
    probe = nc.sync.nop(nofuse=True, hint="drain_wait_spread")
    wait_clock.add_sem_waits(probe.ins, ScopedClock({None: tick_clock.global_clock}))
    si = probe.ins.sync_info
    waits = list(si.on_wait) if si is not None else []
    if len(waits) > _MAX_WAITS_PER_INST:
        probe.ins.sync_info = mybir.SyncInfo(
            on_wait=waits[:_MAX_WAITS_PER_INST], on_update=[]
        )
        for i in range(_MAX_WAITS_PER_INST, len(waits), _MAX_WAITS_PER_INST):
            nop = nc.sync.nop(nofuse=True, hint="drain_wait_spread")
            nop.ins.sync_info = mybir.SyncInfo(
                on_wait=waits[i : i + _MAX_WAITS_PER_INST], on_update=[]
            )
    nc.sync.drain()
    nc.all_engine_barrier(sem_only=True)
    popped = nc._tile_sem_poison_stack.pop()
    assert popped is self._sem_poison
    nc.clear_and_free_semaphores(list(self.sems.allocated().values()))


tile.TileContext._drain_and_barrier = _patched_drain_and_barrier


def _split_multi_waits(nc, max_waits=1):
    """walrus rejects instructions with more than one sem wait; move the
    excess onto same-engine NoOps placed immediately before."""
    ctr = 0
    for blk in nc.m.functions[0].blocks:
        out = []
        for inst in blk.instructions:
            si = inst.sync_info
            waits = list(si.on_wait) if (si and si.on_wait) else []
            if len(waits) > max_waits:
                extra, keep = waits[:-max_waits], waits[-max_waits:]
                for j in range(0, len(extra), max_waits):
                    ctr += 1
                    nop = mybir.InstNoOp(name=f"I-wsplit-{ctr}")
                    nop.engine = inst.engine
                    nop.sync_info = mybir.SyncInfo(
                        on_wait=extra[j : j + max_waits], on_update=[])
                    out.append(nop)
                inst.sync_info = mybir.SyncInfo(
                    on_wait=keep,
                    on_update=list(si.on_update) if si.on_update else [])
            out.append(inst)
        blk.instructions = out
    return ctr
# ---------------------------------------------------------------------------

B = 4
C = 512
H = W = 32
HWF = 1024  # keys / full spatial
Q = 512  # queries per core (half of HWF)
NH = 8
CHD = 64  # channels per head
CT = 4  # 128-channel tiles of C
KT = 8  # 128-key tiles of HWF
GROUPS = 32
GPC = 16  # channels per group
EPS = 1e-6
F32 = mybir.dt.float32

_DT_NAME = os.environ.get("BASS_ATTN_DT", "bf16")
DT = {"f32": mybir.dt.float32, "bf16": mybir.dt.bfloat16,
      "f32r": mybir.dt.float32r}[_DT_NAME]


def _mm(x):  # AP view fed to the tensor engine
    return x


def build_program():
    nc = bass.Bass("TRN2", target_bir_lowering=False, debug=False, num_devices=8)

    def din(name, shape, dt=F32):
        return nc.declare_dram_parameter(name, list(shape), dt, isOutput=False)

    xs_d = din("xs", [C, Q])
    xo_d = din("xo", [C, Q], mybir.dt.bfloat16)
    kvf_d = din("kvf", [C, HWF], mybir.dt.bfloat16)
    wq_d = din("wqT", [C, C], DT)
    wk_d = din("wkT", [C, C], DT)
    wv_d = din("wvT", [C, C], DT)
    wp_d = din("wpT", [C, C], DT)
    bv_d = din("bv", [C])
    cpack_d = din("cpack", [128, 36])
    e16_d = din("e16", [8, 128])
    eh8_d = din("eh8", [8, 512], DT)
    eh2_d = din("eh2", [2, 128], DT)
    out_d = nc.declare_dram_parameter("out", [C, Q], F32, isOutput=True)

    from contextlib import ExitStack
    with tile.TileContext(nc) as tc, ExitStack() as ctx:
        cst = ctx.enter_context(tc.tile_pool(name="cst", bufs=1))
        big = ctx.enter_context(tc.tile_pool(name="big", bufs=1))
        wrk = ctx.enter_context(tc.tile_pool(name="wrk", bufs=2))
        epool = ctx.enter_context(tc.tile_pool(name="epool", bufs=8))
        ps_s = ctx.enter_context(tc.tile_pool(name="ps_s", bufs=2, space="PSUM"))
        ps_o = ctx.enter_context(tc.tile_pool(name="ps_o", bufs=1, space="PSUM"))
        ps_mm = ctx.enter_context(tc.tile_pool(name="ps_mm", bufs=2, space="PSUM"))

        def dma_split(out_ap, in_ap, nsplit=4):
            p = out_ap.shape[0]
            step = p // nsplit
            for i in range(nsplit):
                nc.sync.dma_start(out_ap[i * step : (i + 1) * step],
                                  in_ap[i * step : (i + 1) * step])

        # ---- big inputs (emitted first so kvf owns the DMA queues) ----
        xs = []
        xo = []
        kvf = []
        for t in range(CT):
            halves = []
            for h in range(2):
                kh = big.tile([128, 512], mybir.dt.bfloat16, name=f"kvf{t}_{h}")
                nc.sync.dma_start(
                    kh[:],
                    kvf_d[:].rearrange("(m p) (h q) -> m p h q", p=128, h=2)[t, :, h])
                halves.append(kh)
            kvf.append(halves)
        for t in range(CT):
            xst = big.tile([128, Q], F32, name=f"xs{t}")
            nc.sync.dma_start(xst[:], xs_d[:].rearrange("(m p) q -> m p q", p=128)[t])
            xs.append(xst)
            xot = big.tile([128, Q], mybir.dt.bfloat16, name=f"xo{t}")
            nc.sync.dma_start(xot[:], xo_d[:].rearrange("(m p) q -> m p q", p=128)[t])
            xo.append(xot)

        wq_sb, wk_sb, wv_sb, wp_sb = [], [], [], []
        for wd, lst in ((wq_d, wq_sb), (wk_d, wk_sb), (wv_d, wv_sb), (wp_d, wp_sb)):
            for k in range(CT):
                t_ = big.tile([128, C], DT, name=f"w_{wd.name}{k}")
                nc.sync.dma_start(t_[:], wd[:].rearrange("(k p) m -> k p m", p=128)[k])
                lst.append(t_)

        # ---- constants / small inputs ----
        cpk = cst.tile([128, 36], F32)
        nc.sync.dma_start(cpk[:], cpack_d[:])
        bq_c, bk_c, bp_c = cpk[:, 0:4], cpk[:, 4:8], cpk[:, 8:12]
        gqs_c, gqb_c = cpk[:, 12:16], cpk[:, 16:20]
        gks_c, gkb_c = cpk[:, 20:24], cpk[:, 24:28]
        g16 = cpk[:, 28:36]
        e16 = cst.tile([8, 128], F32)
        nc.sync.dma_start(e16[:], e16_d[:])
        eh8 = cst.tile([8, 512], DT)
        nc.sync.dma_start(eh8[:], eh8_d[:])
        eh2 = cst.tile([2, 128], DT)
        nc.sync.dma_start(eh2[:], eh2_d[:])
        bv_ap = bv_d[:]
        bvbc = cst.tile([128, C], F32)
        nc.gpsimd.dma_start(
            out=bvbc[:],
            in_=bass.AP(tensor=bv_ap.tensor, offset=bv_ap.offset,
                        ap=[[0, 128]] + list(bv_ap.ap)),
        )

        # ---- groupnorm affine coefficients (a, b per channel) ----
        def gn_coeffs(src_chunks, gam, bet, label):
            statc = wrk.tile([128, 8], F32, name=f"statc_{label}", bufs=1)
            for t in range(CT):
                bnst = wrk.tile([128, 2, 6], F32, name=f"bnst_{label}", tag="bnst")
                for half, chunk in enumerate(src_chunks[t]):
                    nc.vector.bn_stats(out=bnst[:, half, :], in_=chunk)
                mv = wrk.tile([128, 2], F32, name=f"mv_{label}", tag="mv")
                nc.vector.bn_aggr(out=mv[:], in_=bnst[:])
                nc.vector.tensor_copy(statc[:, t : t + 1], mv[:, 0:1])
                msq = wrk.tile([128, 1], F32, name=f"msq_{label}", tag="msq")
                nc.vector.tensor_mul(msq[:], mv[:, 0:1], mv[:, 0:1])
                nc.vector.tensor_add(statc[:, 4 + t : 5 + t], msq[:], mv[:, 1:2])
            gps = ps_mm.tile([128, 512], F32, name=f"gps_{label}", tag="mm")
            nc.tensor.matmul(gps[0:8, 0:8], lhsT=g16, rhs=statc[:],
                             start=True, stop=True)
            gs = wrk.tile([8, 8], F32, name=f"gs_{label}", tag="gs")
            nc.vector.tensor_copy(gs[:], gps[0:8, 0:8])
            ms = wrk.tile([8, 8], F32, name=f"ms_{label}", tag="ms")
            nc.vector.tensor_scalar_mul(ms[:], gs[:], 1.0 / GPC)
            msq8 = wrk.tile([8, 4], F32, name=f"msq8_{label}", tag="msq8")
            nc.vector.tensor_mul(msq8[:], ms[:, 0:4], ms[:, 0:4])
            var8 = wrk.tile([8, 4], F32, name=f"var8_{label}", tag="var8")
            nc.vector.tensor_sub(var8[:], ms[:, 4:8], msq8[:])
            # rstd = exp(-0.5*ln(var+eps)) — keeps ACT on one table set
            lnv = wrk.tile([8, 4], F32, name=f"lnv_{label}", tag="lnv")
            eps8 = wrk.tile([8, 1], F32, name=f"eps8_{label}", tag="eps8")
            nc.vector.memset(eps8[:], EPS)
            nc.scalar.activation(lnv[:], var8[:],
                                 mybir.ActivationFunctionType.Ln, bias=eps8[:])
            rhs2 = wrk.tile([8, 8], F32, name=f"rhs2_{label}", tag="rhs2", bufs=1)
            nc.scalar.activation(rhs2[:, 0:4], lnv[:],
                                 mybir.ActivationFunctionType.Exp, scale=-0.5)
            nc.vector.tensor_copy(rhs2[:, 4:8], ms[:, 0:4])
            pcs = ps_mm.tile([128, 512], F32, name=f"pcs_{label}", tag="mm")
            nc.tensor.matmul(pcs[:, 0:8], lhsT=e16[:], rhs=rhs2[:],
                             start=True, stop=True)
            pc = wrk.tile([128, 8], F32, name=f"pc_{label}", tag="pc")
            nc.vector.tensor_copy(pc[:], pcs[:, 0:8])
            a = wrk.tile([128, 4], F32, name=f"a_{label}", bufs=1)
            nc.vector.tensor_mul(a[:], pc[:, 0:4], gam)
            tmpb = wrk.tile([128, 4], F32, name=f"tmpb_{label}", tag="tmpb")
            nc.vector.tensor_mul(tmpb[:], pc[:, 4:8], a[:])
            b = wrk.tile([128, 4], F32, name=f"b_{label}", bufs=1)
            nc.vector.tensor_sub(b[:], bet, tmpb[:])
            return a, b

        akv, bkv = gn_coeffs(
            [(kvf[t][0][:], kvf[t][1][:]) for t in range(CT)],
            gks_c, gkb_c, "kv")

        kvn = []
        for t in range(CT):
            halves = []
            for h in range(2):
                kh = big.tile([128, 512], DT, name=f"kvn{t}_{h}")
                nc.vector.tensor_scalar(
                    out=kh[:], in0=kvf[t][h][:],
                    scalar1=akv[:, t : t + 1], scalar2=bkv[:, t : t + 1],
                    op0=mybir.AluOpType.mult, op1=mybir.AluOpType.add)
                halves.append(kh)
            kvn.append(halves)

        def qkv_ps(i, name):
            r = i % 4
            if r == 2:
                return ps_o.tile([128, 512], F32, name=name, tag="oA")
            if r == 3:
                return ps_o.tile([128, 512], F32, name=name, tag="oB")
            return ps_mm.tile([128, 512], F32, name=name, tag="mm")

        ax, bx = gn_coeffs([(xs[t][:], xo[t][:]) for t in range(CT)],
                           gqs_c, gqb_c, "x")
        qin = []
        for t in range(CT):
            qt = big.tile([128, Q], DT, name=f"qin{t}")
            nc.vector.tensor_scalar(
                out=qt[:], in0=xs[t][:],
                scalar1=ax[:, t : t + 1], scalar2=bx[:, t : t + 1],
                op0=mybir.AluOpType.mult, op1=mybir.AluOpType.add)
            qin.append(qt)

        k_sb = [None] * CT
        q_sb = [None] * CT

        def emit_k(m):
            kt_ = big.tile([128, HWF], DT, name=f"k{m}")
            for nh in range(2):
                ps = qkv_ps(2 * m + nh, f"psk{m}{nh}")
                for k in range(CT):
                    nc.tensor.matmul(
                        ps[:], lhsT=_mm(wk_sb[k][:, bass.ts(m, 128)]),
                        rhs=_mm(kvn[k][nh][:]),
                        start=(k == 0), stop=(k == CT - 1))
                nc.scalar.activation(kt_[:, bass.ts(nh, 512)], ps[:],
                                     mybir.ActivationFunctionType.Identity,
                                     bias=bk_c[:, m : m + 1])
            k_sb[m] = kt_

        def emit_q(m):
            ps = qkv_ps(m, f"psq{m}")
            for k in range(CT):
                nc.tensor.matmul(ps[:], lhsT=_mm(wq_sb[k][:, bass.ts(m, 128)]),
                                 rhs=_mm(qin[k][:]), start=(k == 0),
                                 stop=(k == CT - 1))
            qt = big.tile([128, Q], DT, name=f"q{m}")
            nc.scalar.activation(qt[:], ps[:],
                                 mybir.ActivationFunctionType.Identity,
                                 bias=bq_c[:, m : m + 1])
            q_sb[m] = qt

        emit_k(0)
        emit_q(0)

        vT_sb = [None] * KT

        def emit_v(mt):
            vt = big.tile([128, NH * (CHD + 1)], DT, name=f"vT{mt}")
            ones_col = vt[:].rearrange("p (h c) -> p h c", c=CHD + 1)[
                :, :, CHD : CHD + 1]
            if DT == mybir.dt.float32r:
                ones_col = ones_col.bitcast(F32)
            nc.vector.memset(ones_col, 1.0)
            ps = qkv_ps(mt, f"psv{mt}")
            for k in range(CT):
                nc.tensor.matmul(
                    ps[:], lhsT=_mm(kvn[k][mt // 4][:, bass.ts(mt % 4, 128)]),
                    rhs=_mm(wv_sb[k][:]), start=(k == 0), stop=(k == CT - 1))
            nc.vector.tensor_tensor(
                out=vt[:].rearrange("p (h c) -> p h c", c=CHD + 1)[:, :, 0:CHD],
                in0=ps[:].rearrange("p (h c) -> p h c", c=CHD),
                in1=bvbc[:].rearrange("p (h c) -> p h c", c=CHD),
                op=mybir.AluOpType.add)
            vT_sb[mt] = vt

        emit_v(0)
        emit_v(1)
        for m in range(1, CT):
            emit_k(m)
        for m in range(1, CT):
            emit_q(m)
        for mt in range(2, KT):
            emit_v(mt)

        # ---- attention (head pairs t: heads 2t partitions 0:64, 2t+1 64:128)
        on_sb = [None] * CT
        rz_early = wrk.tile([6, 512], F32, name="rz_early", bufs=1)
        rz_late = wrk.tile([2, 512], F32, name="rz_late", bufs=1)
        rzbE = wrk.tile([8, 512], DT, name="rzbE", bufs=1)
        nc.vector.memset(rzbE[:], 0.0)
        osts = []
        for t in range(CT):
            poA = ps_o.tile([128, 512], F32, name=f"poA{t}", tag="oA")
            poB = ps_o.tile([128, 512], F32, name=f"poB{t}", tag="oB")
            for mk in range(KT):
                pss = ps_s.tile([128, 1024], F32, name=f"pss{t}{mk}", tag="s")
                nc.tensor.matmul(pss[:, 0:512],
                                 lhsT=_mm(k_sb[t][0:64, bass.ts(mk, 128)]),
                                 rhs=_mm(q_sb[t][0:64, :]),
                                 start=True, stop=True, tile_position=(0, 0))
                nc.tensor.matmul(pss[:, 512:1024],
                                 lhsT=_mm(k_sb[t][64:128, bass.ts(mk, 128)]),
                                 rhs=_mm(q_sb[t][64:128, :]),
                                 start=True, stop=True, tile_position=(64, 0))
                et = epool.tile([128, 1024], DT, name=f"e{t}{mk}", tag="e")
                nc.scalar.activation(et[:], pss[:],
                                     mybir.ActivationFunctionType.Exp,
                                     scale=float(CHD) ** -0.5)
                nc.tensor.matmul(poA[0:65, :],
                                 lhsT=_mm(vT_sb[mk][:, bass.ds(130 * t, 65)]),
                                 rhs=_mm(et[:, 0:512]),
                                 start=(mk == 0), stop=(mk == KT - 1))
                nc.tensor.matmul(poB[0:65, :],
                                 lhsT=_mm(vT_sb[mk][:, bass.ds(130 * t + 65, 65)]),
                                 rhs=_mm(et[:, 512:1024]),
                                 start=(mk == 0), stop=(mk == KT - 1))
                if t == 3 and mk == 2:
                    lnE = wrk.tile([6, 512], F32, name="lnE", bufs=1)
                    nc.scalar.activation(lnE[:], rz_early[:],
                                         mybir.ActivationFunctionType.Ln)
                    nc.scalar.activation(rzbE[0:6, :], lnE[:],
                                         mybir.ActivationFunctionType.Exp,
                                         scale=-1.0)
            # head A rows + Z_A row staged separately; head B staged for the
            # partition-shift DMA into rows 64:128 of the shared ost tile.
            ost = wrk.tile([128, 512], F32, name=f"ost{t}", tag="ost", bufs=4)
            nc.vector.tensor_copy(ost[0:64, :], poA[0:64, :])
            zst = wrk.tile([65, 512], F32, name=f"zst{t}", tag="zst", bufs=4)
            nc.vector.tensor_copy(zst[64:65, :], poA[64:65, :])
            stB = wrk.tile([128, 512], F32, name=f"stB{t}", tag="stB", bufs=4)
            nc.vector.tensor_copy(stB[0:65, :], poB[0:65, :])
            if t < 3:
                nc.sync.dma_start(rz_early[2 * t : 2 * t + 1, :], zst[64:65, :])
                nc.sync.dma_start(rz_early[2 * t + 1 : 2 * t + 2, :], stB[64:65, :])
            else:
                nc.sync.dma_start(rz_late[0:1, :], zst[64:65, :])
                nc.sync.dma_start(rz_late[1:2, :], stB[64:65, :])
            nc.sync.dma_start(ost[64:128, :], stB[0:64, :])
            osts.append(ost)

        lnL = wrk.tile([2, 512], F32, name="lnL", bufs=1)
        nc.scalar.activation(lnL[:], rz_late[:], mybir.ActivationFunctionType.Ln)
        rzbL = wrk.tile([2, 512], DT, name="rzbL", bufs=1)
        nc.scalar.activation(rzbL[:], lnL[:], mybir.ActivationFunctionType.Exp,
                             scale=-1.0)
        for t in range(CT):
            zps = ps_mm.tile([128, 512], F32, name=f"zps{t}", tag="mm")
            if t < 3:
                nc.tensor.matmul(zps[:], lhsT=eh8[:, bass.ts(t, 128)],
                                 rhs=rzbE[:], start=True, stop=True)
            else:
                nc.tensor.matmul(zps[:], lhsT=eh2[:], rhs=rzbL[:],
                                 start=True, stop=True)
            ont = big.tile([128, Q], DT, name=f"on{t}")
            nc.vector.tensor_mul(ont[:], osts[t][:], zps[:])
            on_sb[t] = ont

        # ---- proj + residual (k-major so chains start as onts arrive) ----
        proj_ps = []
        for m in range(CT):
            if m == 2:
                ps = ps_o.tile([128, 512], F32, name=f"psp{m}", tag="oA")
            elif m == 3:
                ps = ps_o.tile([128, 512], F32, name=f"psp{m}", tag="oB")
            else:
                ps = ps_mm.tile([128, 512], F32, name=f"psp{m}", tag="mm")
            proj_ps.append(ps)
        for k in range(CT):
            for m in range(CT):
                nc.tensor.matmul(proj_ps[m][:],
                                 lhsT=_mm(wp_sb[k][:, bass.ts(m, 128)]),
                                 rhs=_mm(on_sb[k][:]), start=(k == 0),
                                 stop=(k == CT - 1))
        for m in range(CT):
            r1 = wrk.tile([128, Q], F32, name=f"r1_{m}", tag="r1")
            nc.scalar.activation(r1[:], proj_ps[m][:],
                                 mybir.ActivationFunctionType.Identity,
                                 bias=bp_c[:, m : m + 1])
            r2 = wrk.tile([128, Q], F32, name=f"r2_{m}", tag="r2")
            nc.vector.tensor_add(r2[:], r1[:], xs[m][:])
            nc.sync.dma_start(
                out_d[:].rearrange("(m p) q -> m p q", p=128)[m], r2[:])

    _split_multi_waits(nc)
    return nc


_NC_CACHE = None
LAST_EXEC_NS = None


def _np_dt():
    if DT == mybir.dt.bfloat16:
        import ml_dtypes
        return ml_dtypes.bfloat16
    return np.float32


def kernel(**inputs):
    global _NC_CACHE, LAST_EXEC_NS
    x = np.asarray(inputs["x"], dtype=np.float32)
    kv = np.asarray(inputs["kv"], dtype=np.float32)
    wdt = _np_dt()
    wqT = np.ascontiguousarray(np.asarray(inputs["wq"], np.float32).T).astype(wdt)
    wkT = np.ascontiguousarray(np.asarray(inputs["wk"], np.float32).T).astype(wdt)
    wvT = np.ascontiguousarray(np.asarray(inputs["wv"], np.float32).T).astype(wdt)
    wpT = np.ascontiguousarray(np.asarray(inputs["wproj"], np.float32).T).astype(wdt)
    bq = np.asarray(inputs["bq"], np.float32)
    bk = np.asarray(inputs["bk"], np.float32)
    bv = np.asarray(inputs["bv"], np.float32)
    bp = np.asarray(inputs["bproj"], np.float32)
    gqs = np.asarray(inputs["gnq_scale"], np.float32)
    gqb = np.asarray(inputs["gnq_bias"], np.float32)
    gks = np.asarray(inputs["gnkv_scale"], np.float32)
    gkb = np.asarray(inputs["gnkv_bias"], np.float32)

    p = np.arange(128)
    g16 = (p[:, None] // GPC == np.arange(8)[None, :]).astype(np.float32)
    e16 = np.ascontiguousarray(g16.T)
    eh8 = (np.arange(512)[None, :] // CHD == np.arange(8)[:, None]).astype(
        _np_dt())
    eh2 = (np.arange(128)[None, :] // CHD == np.arange(2)[:, None]).astype(
        _np_dt())
    cpack = np.concatenate(
        [v.reshape(4, 128).T for v in (bq, bk, bp, gqs, gqb, gks, gkb)]
        + [g16], axis=1).astype(np.float32)
    cpack = np.ascontiguousarray(cpack)

    xr = x.reshape(B, C, HWF)
    kvr = kv.reshape(B, C, HWF)

    in_maps = []
    for core in range(8):
        b, s = core // 2, core % 2
        import ml_dtypes
        in_maps.append({
            "xs": np.ascontiguousarray(xr[b][:, s * Q : (s + 1) * Q]),
            "xo": np.ascontiguousarray(
                xr[b][:, (1 - s) * Q : (2 - s) * Q]).astype(ml_dtypes.bfloat16),
            "kvf": np.ascontiguousarray(kvr[b]).astype(ml_dtypes.bfloat16),
            "wqT": wqT, "wkT": wkT, "wvT": wvT, "wpT": wpT,
            "bv": bv, "cpack": cpack, "e16": e16, "eh8": eh8, "eh2": eh2,
        })

    if _NC_CACHE is None:
        _NC_CACHE = build_program()

    trace = os.environ.get("BASS_ATTN_TRACE", "0") == "1"
    res = run_bass_kernel_spmd(_NC_CACHE, in_maps, core_ids=list(range(8)),
                               trace=trace)
    LAST_EXEC_NS = res.exec_time_ns

    out = np.empty((B, C, HWF), np.float32)
    for core in range(8):
        b, s = core // 2, core % 2
        out[b][:, s * Q : (s + 1) * Q] = res.results[core]["out"]
    return out.reshape(B, C, H, W)


# revision 42
# speedup vs baseline: 1.0818x; 1.0818x over previous
"""AttnBlock (GroupNorm -> qkv 1x1 conv -> 8-head attention over 32x32
spatial -> proj 1x1 conv -> residual) on 8 Trainium2 NeuronCores.

Sharding: fully data-parallel, no collectives. Core i handles batch
b = i//2 and query-half s = i%2 (512 of the 1024 spatial positions).
Each core redundantly computes GroupNorm stats plus the full k/v
projections for its batch, then scores/softmax/AV/proj for its query
half. Host concatenates the per-core [512, 512] outputs.

Per-core device program (compute dtype bf16, f32 accumulation):
  - GroupNorm stats via bn_stats/bn_aggr per channel, group-reduced
    across partitions with a tiny selector matmul, expanded back with a
    second matmul; affine folded into one tensor_scalar per tile.
  - Scores are computed transposed, S^T[j,i] = sum_c k[c,j] q[c,i]
    (keys on partitions, K=64 per head), with the two heads of a
    partition-tile row-packed into PE row groups (0,0)/(64,0) so a pair
    costs one N=512 stream.
  - softmax skips the max-subtraction (|scores| <= ~6 for this
    problem's GN'd inputs): one Exp activation per [128,1024] psum with
    the 1/sqrt(64) scale folded in.  The key-dim sum comes free from a
    ones-column appended to v^T in the AV matmul (psum row 64 = Z);
    1/Z = exp(-ln Z) on the scalar engine, broadcast across partitions
    with a tiny expander matmul.
  - proj accumulates k-major so chains start as head-pair outputs
    arrive; residual added from the f32 x half kept on chip.

Toolchain workarounds: the Tile-tail Drain and any instruction carrying
more than one semaphore wait are rejected by this walrus build, so
excess waits are spread onto same-engine NoOps post-schedule.
"""

import os

import numpy as np

import concourse.bass as bass
import concourse.tile as tile
from concourse import mybir
from concourse.bass_utils import run_bass_kernel_spmd
from concourse.vector_clock import ScopedClock

# ---------------------------------------------------------------------------
# walrus workaround: the Tile kernel-tail Drain may carry more sem waits than
# the CTRL instruction encoding allows; spread them over sync-engine NOPs.
_MAX_WAITS_PER_INST = 1


def _patched_drain_and_barrier(self, tick_clock, wait_clock):
    nc = self.nc
    probe = nc.sync.nop(nofuse=True, hint="drain_wait_spread")
    wait_clock.add_sem_waits(probe.ins, ScopedClock({None: tick_clock.global_clock}))
    si = probe.ins.sync_info
    waits = list(si.on_wait) if si is not None else []
    if len(waits) > _MAX_WAITS_PER_INST:
        probe.ins.sync_info = mybir.SyncInfo(
            on_wait=waits[:_MAX_WAITS_PER_INST], on_update=[]
        )
        for i in range(_MAX_WAITS_PER_INST, len(waits), _MAX_WAITS_PER_INST):
            nop = nc.sync.nop(nofuse=True, hint="drain_wait_spread")
            nop.ins.sync_info = mybir.SyncInfo(
                on_wait=waits[i : i + _MAX_WAITS_PER_INST], on_update=[]
            )
    nc.sync.drain()
    nc.all_engine_barrier(sem_only=True)
    popped = nc._tile_sem_poison_stack.pop()
    assert popped is self._sem_poison
    nc.clear_and_free_semaphores(list(self.sems.allocated().values()))


tile.TileContext._drain_and_barrier = _patched_drain_and_barrier


def _split_multi_waits(nc, max_waits=1):
    """walrus rejects instructions with more than one sem wait; move the
    excess onto same-engine NoOps placed immediately before."""
    ctr = 0
    for blk in nc.m.functions[0].blocks:
        out = []
        for inst in blk.instructions:
            si = inst.sync_info
            waits = list(si.on_wait) if (si and si.on_wait) else []
            if len(waits) > max_waits:
                extra, keep = waits[:-max_waits], waits[-max_waits:]
                for j in range(0, len(extra), max_waits):
                    ctr += 1
                    nop = mybir.InstNoOp(name=f"I-wsplit-{ctr}")
                    nop.engine = inst.engine
                    nop.sync_info = mybir.SyncInfo(
                        on_wait=extra[j : j + max_waits], on_update=[])
                    out.append(nop)
                inst.sync_info = mybir.SyncInfo(
                    on_wait=keep,
                    on_update=list(si.on_update) if si.on_update else [])
            out.append(inst)
        blk.instructions = out
    return ctr
# ---------------------------------------------------------------------------

B = 4
C = 512
H = W = 32
HWF = 1024  # keys / full spatial
Q = 512  # queries per core (half of HWF)
NH = 8
CHD = 64  # channels per head
CT = 4  # 128-channel tiles of C
KT = 8  # 128-key tiles of HWF
GROUPS = 32
GPC = 16  # channels per group
EPS = 1e-6
F32 = mybir.dt.float32

_DT_NAME = os.environ.get("BASS_ATTN_DT", "bf16")
DT = {"f32": mybir.dt.float32, "bf16": mybir.dt.bfloat16,
      "f32r": mybir.dt.float32r}[_DT_NAME]


def _mm(x):  # AP view fed to the tensor engine
    return x


def build_program():
    nc = bass.Bass("TRN2", target_bir_lowering=False, debug=False, num_devices=8)

    def din(name, shape, dt=F32):
        return nc.declare_dram_parameter(name, list(shape), dt, isOutput=False)

    xs_d = din("xs", [C, Q])
    xo_d = din("xo", [C, Q], mybir.dt.bfloat16)
    kvf_d = din("kvf", [C, HWF], mybir.dt.bfloat16)
    wq_d = din("wqT", [C, C], DT)
    wk_d = din("wkT", [C, C], DT)
    wv_d = din("wvT", [C, C], DT)
    wp_d = din("wpT", [C, C], DT)
    bv_d = din("bv", [C])
    cpack_d = din("cpack", [128, 36])
    e16_d = din("e16", [8, 128])
    eh8_d = din("eh8", [8, 512], DT)
    eh2_d = din("eh2", [2, 128], DT)
    out_d = nc.declare_dram_parameter("out", [C, Q], F32, isOutput=True)

    from contextlib import ExitStack
    with tile.TileContext(nc) as tc, ExitStack() as ctx:
        cst = ctx.enter_context(tc.tile_pool(name="cst", bufs=1))
        big = ctx.enter_context(tc.tile_pool(name="big", bufs=1))
        wrk = ctx.enter_context(tc.tile_pool(name="wrk", bufs=2))
        epool = ctx.enter_context(tc.tile_pool(name="epool", bufs=8))
        ps_s = ctx.enter_context(tc.tile_pool(name="ps_s", bufs=2, space="PSUM"))
        ps_o = ctx.enter_context(tc.tile_pool(name="ps_o", bufs=1, space="PSUM"))
        ps_mm = ctx.enter_context(tc.tile_pool(name="ps_mm", bufs=2, space="PSUM"))

        def dma_split(out_ap, in_ap, nsplit=4):
            p = out_ap.shape[0]
            step = p // nsplit
            for i in range(nsplit):
                nc.sync.dma_start(out_ap[i * step : (i + 1) * step],
                                  in_ap[i * step : (i + 1) * step])

        # ---- constants / small inputs ----
        # cpack: host-prepared [128, 36]: 7 vectors as [128,4] blocks + g16
        cpk = cst.tile([128, 36], F32)
        nc.sync.dma_start(cpk[:], cpack_d[:])
        bq_c, bk_c, bp_c = cpk[:, 0:4], cpk[:, 4:8], cpk[:, 8:12]
        gqs_c, gqb_c = cpk[:, 12:16], cpk[:, 16:20]
        gks_c, gkb_c = cpk[:, 20:24], cpk[:, 24:28]
        g16 = cpk[:, 28:36]
        e16 = cst.tile([8, 128], F32)
        nc.sync.dma_start(e16[:], e16_d[:])
        eh8 = cst.tile([8, 512], DT)
        nc.sync.dma_start(eh8[:], eh8_d[:])
        eh2 = cst.tile([2, 128], DT)
        nc.sync.dma_start(eh2[:], eh2_d[:])
        bv_ap = bv_d[:]
        bvbc = cst.tile([128, C], F32)
        nc.gpsimd.dma_start(
            out=bvbc[:],
            in_=bass.AP(tensor=bv_ap.tensor, offset=bv_ap.offset,
                        ap=[[0, 128]] + list(bv_ap.ap)),
        )

        # ---- big inputs ----
        xs = []
        xo = []
        kvf = []
        for t in range(CT):
            halves = []
            for h in range(2):
                kh = big.tile([128, 512], mybir.dt.bfloat16, name=f"kvf{t}_{h}")
                nc.sync.dma_start(
                    kh[:],
                    kvf_d[:].rearrange("(m p) (h q) -> m p h q", p=128, h=2)[t, :, h])
                halves.append(kh)
            kvf.append(halves)
        for t in range(CT):
            xst = big.tile([128, Q], F32, name=f"xs{t}")
            nc.sync.dma_start(xst[:], xs_d[:].rearrange("(m p) q -> m p q", p=128)[t])
            xs.append(xst)
            xot = big.tile([128, Q], mybir.dt.bfloat16, name=f"xo{t}")
            nc.sync.dma_start(xot[:], xo_d[:].rearrange("(m p) q -> m p q", p=128)[t])
            xo.append(xot)

        wq_sb, wk_sb, wv_sb, wp_sb = [], [], [], []
        for wd, lst in ((wq_d, wq_sb), (wk_d, wk_sb), (wv_d, wv_sb), (wp_d, wp_sb)):
            for k in range(CT):
                t_ = big.tile([128, C], DT, name=f"w_{wd.name}{k}")
                nc.sync.dma_start(t_[:], wd[:].rearrange("(k p) m -> k p m", p=128)[k])
                lst.append(t_)

        # ---- groupnorm affine coefficients (a, b per channel) ----
        def gn_coeffs(src_chunks, gam, bet, label):
            statc = wrk.tile([128, 8], F32, name=f"statc_{label}", bufs=1)
            for t in range(CT):
                bnst = wrk.tile([128, 2, 6], F32, name=f"bnst_{label}", tag="bnst")
                for half, chunk in enumerate(src_chunks[t]):
                    nc.vector.bn_stats(out=bnst[:, half, :], in_=chunk)
                mv = wrk.tile([128, 2], F32, name=f"mv_{label}", tag="mv")
                nc.vector.bn_aggr(out=mv[:], in_=bnst[:])
                nc.vector.tensor_copy(statc[:, t : t + 1], mv[:, 0:1])
                msq = wrk.tile([128, 1], F32, name=f"msq_{label}", tag="msq")
                nc.vector.tensor_mul(msq[:], mv[:, 0:1], mv[:, 0:1])
                nc.vector.tensor_add(statc[:, 4 + t : 5 + t], msq[:], mv[:, 1:2])
            gps = ps_mm.tile([128, 512], F32, name=f"gps_{label}", tag="mm")
            nc.tensor.matmul(gps[0:8, 0:8], lhsT=g16, rhs=statc[:],
                             start=True, stop=True)
            gs = wrk.tile([8, 8], F32, name=f"gs_{label}", tag="gs")
            nc.vector.tensor_copy(gs[:], gps[0:8, 0:8])
            ms = wrk.tile([8, 8], F32, name=f"ms_{label}", tag="ms")
            nc.vector.tensor_scalar_mul(ms[:], gs[:], 1.0 / GPC)
            msq8 = wrk.tile([8, 4], F32, name=f"msq8_{label}", tag="msq8")
            nc.vector.tensor_mul(msq8[:], ms[:, 0:4], ms[:, 0:4])
            var8 = wrk.tile([8, 4], F32, name=f"var8_{label}", tag="var8")
            nc.vector.tensor_sub(var8[:], ms[:, 4:8], msq8[:])
            # rstd = exp(-0.5*ln(var+eps)) — keeps ACT on one table set
            lnv = wrk.tile([8, 4], F32, name=f"lnv_{label}", tag="lnv")
            eps8 = wrk.tile([8, 1], F32, name=f"eps8_{label}", tag="eps8")
            nc.vector.memset(eps8[:], EPS)
            nc.scalar.activation(lnv[:], var8[:],
                                 mybir.ActivationFunctionType.Ln, bias=eps8[:])
            rhs2 = wrk.tile([8, 8], F32, name=f"rhs2_{label}", tag="rhs2", bufs=1)
            nc.scalar.activation(rhs2[:, 0:4], lnv[:],
                                 mybir.ActivationFunctionType.Exp, scale=-0.5)
            nc.vector.tensor_copy(rhs2[:, 4:8], ms[:, 0:4])
            pcs = ps_mm.tile([128, 512], F32, name=f"pcs_{label}", tag="mm")
            nc.tensor.matmul(pcs[:, 0:8], lhsT=e16[:], rhs=rhs2[:],
                             start=True, stop=True)
            pc = wrk.tile([128, 8], F32, name=f"pc_{label}", tag="pc")
            nc.vector.tensor_copy(pc[:], pcs[:, 0:8])
            a = wrk.tile([128, 4], F32, name=f"a_{label}", bufs=1)
            nc.vector.tensor_mul(a[:], pc[:, 0:4], gam)
            tmpb = wrk.tile([128, 4], F32, name=f"tmpb_{label}", tag="tmpb")
            nc.vector.tensor_mul(tmpb[:], pc[:, 4:8], a[:])
            b = wrk.tile([128, 4], F32, name=f"b_{label}", bufs=1)
            nc.vector.tensor_sub(b[:], bet, tmpb[:])
            return a, b

        akv, bkv = gn_coeffs(
            [(kvf[t][0][:], kvf[t][1][:]) for t in range(CT)],
            gks_c, gkb_c, "kv")

        kvn = []
        for t in range(CT):
            halves = []
            for h in range(2):
                kh = big.tile([128, 512], DT, name=f"kvn{t}_{h}")
                nc.vector.tensor_scalar(
                    out=kh[:], in0=kvf[t][h][:],
                    scalar1=akv[:, t : t + 1], scalar2=bkv[:, t : t + 1],
                    op0=mybir.AluOpType.mult, op1=mybir.AluOpType.add)
                halves.append(kh)
            kvn.append(halves)

        def qkv_ps(i, name):
            r = i % 4
            if r == 2:
                return ps_o.tile([128, 512], F32, name=name, tag="oA")
            if r == 3:
                return ps_o.tile([128, 512], F32, name=name, tag="oB")
            return ps_mm.tile([128, 512], F32, name=name, tag="mm")

        ax, bx = gn_coeffs([(xs[t][:], xo[t][:]) for t in range(CT)],
                           gqs_c, gqb_c, "x")
        qin = []
        for t in range(CT):
            qt = big.tile([128, Q], DT, name=f"qin{t}")
            nc.vector.tensor_scalar(
                out=qt[:], in0=xs[t][:],
                scalar1=ax[:, t : t + 1], scalar2=bx[:, t : t + 1],
                op0=mybir.AluOpType.mult, op1=mybir.AluOpType.add)
            qin.append(qt)

        k_sb = [None] * CT
        q_sb = [None] * CT

        def emit_k(m):
            kt_ = big.tile([128, HWF], DT, name=f"k{m}")
            for nh in range(2):
                ps = qkv_ps(2 * m + nh, f"psk{m}{nh}")
                for k in range(CT):
                    nc.tensor.matmul(
                        ps[:], lhsT=_mm(wk_sb[k][:, bass.ts(m, 128)]),
                        rhs=_mm(kvn[k][nh][:]),
                        start=(k == 0), stop=(k == CT - 1))
                nc.scalar.activation(kt_[:, bass.ts(nh, 512)], ps[:],
                                     mybir.ActivationFunctionType.Identity,
                                     bias=bk_c[:, m : m + 1])
            k_sb[m] = kt_

        def emit_q(m):
            ps = qkv_ps(m, f"psq{m}")
            for k in range(CT):
                nc.tensor.matmul(ps[:], lhsT=_mm(wq_sb[k][:, bass.ts(m, 128)]),
                                 rhs=_mm(qin[k][:]), start=(k == 0),
                                 stop=(k == CT - 1))
            qt = big.tile([128, Q], DT, name=f"q{m}")
            nc.scalar.activation(qt[:], ps[:],
                                 mybir.ActivationFunctionType.Identity,
                                 bias=bq_c[:, m : m + 1])
            q_sb[m] = qt

        emit_k(0)
        emit_q(0)

        vT_sb = [None] * KT

        def emit_v(mt):
            vt = big.tile([128, NH * (CHD + 1)], DT, name=f"vT{mt}")
            ones_col = vt[:].rearrange("p (h c) -> p h c", c=CHD + 1)[
                :, :, CHD : CHD + 1]
            if DT == mybir.dt.float32r:
                ones_col = ones_col.bitcast(F32)
            nc.vector.memset(ones_col, 1.0)
            ps = qkv_ps(mt, f"psv{mt}")
            for k in range(CT):
                nc.tensor.matmul(
                    ps[:], lhsT=_mm(kvn[k][mt // 4][:, bass.ts(mt % 4, 128)]),
                    rhs=_mm(wv_sb[k][:]), start=(k == 0), stop=(k == CT - 1))
            nc.vector.tensor_tensor(
                out=vt[:].rearrange("p (h c) -> p h c", c=CHD + 1)[:, :, 0:CHD],
                in0=ps[:].rearrange("p (h c) -> p h c", c=CHD),
                in1=bvbc[:].rearrange("p (h c) -> p h c", c=CHD),
                op=mybir.AluOpType.add)
            vT_sb[mt] = vt

        emit_v(0)
        emit_v(1)
        for m in range(1, CT):
            emit_k(m)
        for m in range(1, CT):
            emit_q(m)
        for mt in range(2, KT):
            emit_v(mt)

        # ---- attention (head pairs t: heads 2t partitions 0:64, 2t+1 64:128)
        on_sb = [None] * CT
        rz_early = wrk.tile([6, 512], F32, name="rz_early", bufs=1)
        rz_late = wrk.tile([2, 512], F32, name="rz_late", bufs=1)
        rzbE = wrk.tile([8, 512], DT, name="rzbE", bufs=1)
        nc.vector.memset(rzbE[:], 0.0)
        osts = []
        for t in range(CT):
            poA = ps_o.tile([128, 512], F32, name=f"poA{t}", tag="oA")
            poB = ps_o.tile([128, 512], F32, name=f"poB{t}", tag="oB")
            for mk in range(KT):
                pss = ps_s.tile([128, 1024], F32, name=f"pss{t}{mk}", tag="s")
                nc.tensor.matmul(pss[:, 0:512],
                                 lhsT=_mm(k_sb[t][0:64, bass.ts(mk, 128)]),
                                 rhs=_mm(q_sb[t][0:64, :]),
                                 start=True, stop=True, tile_position=(0, 0))
                nc.tensor.matmul(pss[:, 512:1024],
                                 lhsT=_mm(k_sb[t][64:128, bass.ts(mk, 128)]),
                                 rhs=_mm(q_sb[t][64:128, :]),
                                 start=True, stop=True, tile_position=(64, 0))
                et = epool.tile([128, 1024], DT, name=f"e{t}{mk}", tag="e")
                nc.scalar.activation(et[:], pss[:],
                                     mybir.ActivationFunctionType.Exp,
                                     scale=float(CHD) ** -0.5)
                nc.tensor.matmul(poA[0:65, :],
                                 lhsT=_mm(vT_sb[mk][:, bass.ds(130 * t, 65)]),
                                 rhs=_mm(et[:, 0:512]),
                                 start=(mk == 0), stop=(mk == KT - 1))
                nc.tensor.matmul(poB[0:65, :],
                                 lhsT=_mm(vT_sb[mk][:, bass.ds(130 * t + 65, 65)]),
                                 rhs=_mm(et[:, 512:1024]),
                                 start=(mk == 0), stop=(mk == KT - 1))
                if t == 3 and mk == 2:
                    lnE = wrk.tile([6, 512], F32, name="lnE", bufs=1)
                    nc.scalar.activation(lnE[:], rz_early[:],
                                         mybir.ActivationFunctionType.Ln)
                    nc.scalar.activation(rzbE[0:6, :], lnE[:],
                                         mybir.ActivationFunctionType.Exp,
                                         scale=-1.0)
            # head A rows + Z_A row staged separately; head B staged for the
            # partition-shift DMA into rows 64:128 of the shared ost tile.
            ost = wrk.tile([128, 512], F32, name=f"ost{t}", tag="ost", bufs=4)
            nc.vector.tensor_copy(ost[0:64, :], poA[0:64, :])
            zst = wrk.tile([65, 512], F32, name=f"zst{t}", tag="zst", bufs=4)
            nc.vector.tensor_copy(zst[64:65, :], poA[64:65, :])
            stB = wrk.tile([128, 512], F32, name=f"stB{t}", tag="stB", bufs=4)
            nc.vector.tensor_copy(stB[0:65, :], poB[0:65, :])
            if t < 3:
                nc.sync.dma_start(rz_early[2 * t : 2 * t + 1, :], zst[64:65, :])
                nc.sync.dma_start(rz_early[2 * t + 1 : 2 * t + 2, :], stB[64:65, :])
            else:
                nc.sync.dma_start(rz_late[0:1, :], zst[64:65, :])
                nc.sync.dma_start(rz_late[1:2, :], stB[64:65, :])
            nc.sync.dma_start(ost[64:128, :], stB[0:64, :])
            osts.append(ost)

        lnL = wrk.tile([2, 512], F32, name="lnL", bufs=1)
        nc.scalar.activation(lnL[:], rz_late[:], mybir.ActivationFunctionType.Ln)
        rzbL = wrk.tile([2, 512], DT, name="rzbL", bufs=1)
        nc.scalar.activation(rzbL[:], lnL[:], mybir.ActivationFunctionType.Exp,
                             scale=-1.0)
        for t in range(CT):
            zps = ps_mm.tile([128, 512], F32, name=f"zps{t}", tag="mm")
            if t < 3:
                nc.tensor.matmul(zps[:], lhsT=eh8[:, bass.ts(t, 128)],
                                 rhs=rzbE[:], start=True, stop=True)
            else:
                nc.tensor.matmul(zps[:], lhsT=eh2[:], rhs=rzbL[:],
                                 start=True, stop=True)
            ont = big.tile([128, Q], DT, name=f"on{t}")
            nc.vector.tensor_mul(ont[:], osts[t][:], zps[:])
            on_sb[t] = ont

        # ---- proj + residual (k-major so chains start as onts arrive) ----
        proj_ps = []
        for m in range(CT):
            if m == 2:
                ps = ps_o.tile([128, 512], F32, name=f"psp{m}", tag="oA")
            elif m == 3:
                ps = ps_o.tile([128, 512], F32, name=f"psp{m}", tag="oB")
            else:
                ps = ps_mm.tile([128, 512], F32, name=f"psp{m}", tag="mm")
            proj_ps.append(ps)
        for k in range(CT):
            for m in range(CT):
                nc.tensor.matmul(proj_ps[m][:],
                                 lhsT=_mm(wp_sb[k][:, bass.ts(m, 128)]),
                                 rhs=_mm(on_sb[k][:]), start=(k == 0),
                                 stop=(k == CT - 1))
        for m in range(CT):
            r1 = wrk.tile([128, Q], F32, name=f"r1_{m}", tag="r1")
            nc.scalar.activation(r1[:], proj_ps[m][:],
                                 mybir.ActivationFunctionType.Identity,
                                 bias=bp_c[:, m : m + 1])
            r2 = wrk.tile([128, Q], F32, name=f"r2_{m}", tag="r2")
            nc.vector.tensor_add(r2[:], r1[:], xs[m][:])
            nc.sync.dma_start(
                out_d[:].rearrange("(m p) q -> m p q", p=128)[m], r2[:])

    _split_multi_waits(nc)
    return nc


_NC_CACHE = None
LAST_EXEC_NS = None


def _np_dt():
    if DT == mybir.dt.bfloat16:
        import ml_dtypes
        return ml_dtypes.bfloat16
    return np.float32


def kernel(**inputs):
    global _NC_CACHE, LAST_EXEC_NS
    x = np.asarray(inputs["x"], dtype=np.float32)
    kv = np.asarray(inputs["kv"], dtype=np.float32)
    wdt = _np_dt()
    wqT = np.ascontiguousarray(np.asarray(inputs["wq"], np.float32).T).astype(wdt)
    wkT = np.ascontiguousarray(np.asarray(inputs["wk"], np.float32).T).astype(wdt)
    wvT = np.ascontiguousarray(np.asarray(inputs["wv"], np.float32).T).astype(wdt)
    wpT = np.ascontiguousarray(np.asarray(inputs["wproj"], np.float32).T).astype(wdt)
    bq = np.asarray(inputs["bq"], np.float32)
    bk = np.asarray(inputs["bk"], np.float32)
    bv = np.asarray(inputs["bv"], np.float32)
    bp = np.asarray(inputs["bproj"], np.float32)
    gqs = np.asarray(inputs["gnq_scale"], np.float32)
    gqb = np.asarray(inputs["gnq_bias"], np.float32)
    gks = np.asarray(inputs["gnkv_scale"], np.float32)
    gkb = np.asarray(inputs["gnkv_bias"], np.float32)

    p = np.arange(128)
    g16 = (p[:, None] // GPC == np.arange(8)[None, :]).astype(np.float32)
    e16 = np.ascontiguousarray(g16.T)
    eh8 = (np.arange(512)[None, :] // CHD == np.arange(8)[:, None]).astype(
        _np_dt())
    eh2 = (np.arange(128)[None, :] // CHD == np.arange(2)[:, None]).astype(
        _np_dt())
    cpack = np.concatenate(
        [v.reshape(4, 128).T for v in (bq, bk, bp, gqs, gqb, gks, gkb)]
        + [g16], axis=1).astype(np.float32)
    cpack = np.ascontiguousarray(cpack)

    xr = x.reshape(B, C, HWF)
    kvr = kv.reshape(B, C, HWF)

    in_maps = []
    for core in range(8):
        b, s = core // 2, core % 2
        import ml_dtypes
        in_maps.append({
            "xs": np.ascontiguousarray(xr[b][:, s * Q : (s + 1) * Q]),
            "xo": np.ascontiguousarray(
                xr[b][:, (1 - s) * Q : (2 - s) * Q]).astype(ml_dtypes.bfloat16),
            "kvf": np.ascontiguousarray(kvr[b]).astype(ml_dtypes.bfloat16),
            "wqT": wqT, "wkT": wkT, "wvT": wvT, "wpT": wpT,
            "bv": bv, "cpack": cpack, "e16": e16, "eh8": eh8, "eh2": eh2,
        })

    if _NC_CACHE is None:
        _NC_CACHE = build_program()

    trace = os.environ.get("BASS_ATTN_TRACE", "0") == "1"
    res = run_bass_kernel_spmd(_NC_CACHE, in_maps, core_ids=list(range(8)),
                               trace=trace)
    LAST_EXEC_NS = res.exec_time_ns

    out = np.empty((B, C, HWF), np.float32)
    for core in range(8):
        b, s = core // 2, core % 2
        out[b][:, s * Q : (s + 1) * Q] = res.results[core]["out"]
    return out.reshape(B, C, H, W)
